# revision 1
# baseline (speedup 1.0000x reference)
"""Context-parallel causal attention block on 8 Trainium2 NeuronCores.

Strategy: tensor-parallel split-heads. Each core c computes Q/K/V projections
for its 2 heads (of 16) over all tokens with host-sliced weights, runs causal
attention locally (feature-major layouts, no transposes), then one on-device
AllToAll re-shards from head-parallel to token-parallel, and each core runs the
output projection for its 512-token row slice. Host concatenates row slices.

Matmul operands are bf16 (fp32 matmuls are two-pass / half-rate on TRN2's PE);
all accumulation stays fp32 in PSUM, softmax runs on fp32 scores.
"""
import sys

sys.path.insert(0, "/opt/trn_rl_repo")

import ml_dtypes
import numpy as np

import concourse.bass as bass
import concourse.tile as tile
from concourse import bacc, mybir
from concourse.bass_utils import run_bass_kernel_spmd

FP = mybir.dt.float32
BF = mybir.dt.bfloat16
NPBF = ml_dtypes.bfloat16
N_CORES = 8
B, S, D, H, DH = 2, 2048, 2048, 16, 128
T = B * S            # 4096 flattened tokens, b-major
KK = D // 128        # 16 contraction k-tiles
NSTRIP = T // 512    # 8 token strips of 512
ROWS = T // N_CORES  # 512 output rows per core
NEG = -1.0e30


def build_nc(debug_taps: bool = False) -> bacc.Bacc:
    nc = bacc.Bacc("TRN2", target_bir_lowering=False, debug=False, num_devices=N_CORES)

    xt = nc.dram_tensor("xt", [128, KK, T], BF, kind="ExternalInput")
    wq = nc.dram_tensor("wq", [128, KK, 256], BF, kind="ExternalInput")
    wk = nc.dram_tensor("wk", [128, KK, 256], BF, kind="ExternalInput")
    wv = nc.dram_tensor("wv", [128, KK, 256], BF, kind="ExternalInput")
    wo = nc.dram_tensor("wo", [128, KK, D], BF, kind="ExternalInput")
    out_t = nc.dram_tensor("out_t", [D, ROWS], FP, kind="ExternalOutput")
    dbg = {}
    if debug_taps:
        dbg["q"] = nc.dram_tensor("dbg_q", [128, 2, T], BF, kind="ExternalOutput")
        dbg["k"] = nc.dram_tensor("dbg_k", [128, 2, T], BF, kind="ExternalOutput")
        dbg["v"] = nc.dram_tensor("dbg_v", [128, 32, 256], BF, kind="ExternalOutput")
        dbg["ain"] = nc.dram_tensor("dbg_ain", [N_CORES, 256, 512], BF, kind="ExternalOutput")
        dbg["aout"] = nc.dram_tensor("dbg_aout", [N_CORES, 256, 512], BF, kind="ExternalOutput")

    with tile.TileContext(nc) as tc:
        with (
            tc.tile_pool(name="dram", bufs=1, space="DRAM") as dram,
            tc.tile_pool(name="consts", bufs=1) as consts,
            tc.tile_pool(name="persist", bufs=1) as persist,
        ):
            a2a_in = [dram.tile([N_CORES, 128, 512], BF, name=f"a2a_in{h}") for h in range(2)]
            a2a_out = [dram.tile([N_CORES, 128, 512], BF, name=f"a2a_out{h}") for h in range(2)]

            ones = consts.tile([128, 1], BF)
            nc.gpsimd.memset(ones[:], 1.0)
            # additive causal masks for the 4 diagonal offsets:
            # masks[p, i, q] = 0 if q >= p + i*128 else NEG
            masks = consts.tile([128, 4, 512], FP)
            nc.gpsimd.memset(masks[:], 0.0)
            for i in range(4):
                nc.gpsimd.affine_select(
                    out=masks[:, i, :],
                    in_=masks[:, i, :],
                    compare_op=mybir.AluOpType.is_ge,
                    fill=NEG,
                    base=-(i * 128),
                    pattern=[[1, 512]],
                    channel_multiplier=-1,
                )

            qT = persist.tile([128, 2, T], BF)       # [dh, hl, t]
            kT = persist.tile([128, 2, T], BF)
            v_sb = persist.tile([128, 32, 256], BF)  # [t%128, t//128, head_feat]

            # ------- Phases 1+2 interleaved: projections + attention -------
            with tc.tile_pool(name="wop", bufs=8) as wop:
                wo_tiles = []

                with (
                    tc.tile_pool(name="wpool", bufs=1) as wpool,
                    tc.tile_pool(name="xtp", bufs=6) as xtp,
                    tc.tile_pool(name="ps1", bufs=2, space="PSUM") as ps1,
                    tc.tile_pool(name="expp", bufs=3) as expp,
                    tc.tile_pool(name="smallp", bufs=2) as smallp,
                    tc.tile_pool(name="otp", bufs=3) as otp,
                    tc.tile_pool(name="psT", bufs=2, space="PSUM") as psT,
                    tc.tile_pool(name="psA", bufs=1, space="PSUM") as psA,
                    tc.tile_pool(name="psS", bufs=1, space="PSUM") as psS,
                ):
                    wq_sb = wpool.tile([128, KK, 256], BF)
                    wk_sb = wpool.tile([128, KK, 256], BF)
                    wv_sb = wpool.tile([128, KK, 256], BF)
                    nc.sync.dma_start(wq_sb[:], wq[:])
                    nc.sync.dma_start(wk_sb[:], wk[:])
                    nc.sync.dma_start(wv_sb[:], wv[:])

                    for b in range(B):
                        if b == 1:
                            # prefetch output-projection weights during batch-1
                            # compute (after batch-0's critical-path loads)
                            for dd in range(KK):
                                wod = wop.tile([128, KK, 128], BF, tag="wod", name=f"wod{dd}")
                                nc.sync.dma_start(wod[:], wo[:, :, dd * 128 : (dd + 1) * 128])
                                wo_tiles.append(wod)
                        for s in range(4):
                            strip = b * 4 + s
                            t0 = strip * 512
                            xq = []
                            for qtr in range(4):
                                xtile = xtp.tile([128, 4, 512], BF, tag="xt")
                                nc.sync.dma_start(
                                    xtile[:],
                                    xt[:, qtr * 4 : (qtr + 1) * 4, t0 : t0 + 512],
                                )
                                xq.append(xtile)

                            # pass A: q for both heads (2 banks)
                            pa = [ps1.tile([128, 512], FP, tag=f"p1{j}", name=f"pa{j}")
                                  for j in range(2)]
                            for kk in range(KK):
                                xsl = xq[kk // 4][:, kk % 4, :]
                                st, sp = kk == 0, kk == KK - 1
                                nc.tensor.matmul(pa[0][:], wq_sb[:, kk, 0:128], xsl, start=st, stop=sp)
                                nc.tensor.matmul(pa[1][:], wq_sb[:, kk, 128:256], xsl, start=st, stop=sp)
                            for hl in range(2):
                                nc.scalar.copy(qT[:, hl, t0 : t0 + 512], pa[hl][:])
                            # pass B: k for both heads
                            pb = [ps1.tile([128, 512], FP, tag=f"p1{j}", name=f"pb{j}")
                                  for j in range(2)]
                            for kk in range(KK):
                                xsl = xq[kk // 4][:, kk % 4, :]
                                st, sp = kk == 0, kk == KK - 1
                                nc.tensor.matmul(pb[0][:], wk_sb[:, kk, 0:128], xsl, start=st, stop=sp)
                                nc.tensor.matmul(pb[1][:], wk_sb[:, kk, 128:256], xsl, start=st, stop=sp)
                            for hl in range(2):
                                nc.scalar.copy(kT[:, hl, t0 : t0 + 512], pb[hl][:])
                            # pass C/D: v in two tt sub-passes (2 banks each)
                            for half in range(2):
                                pv = [ps1.tile([128, 256], FP, tag=f"p1{j}", name=f"pv{j}")
                                      for j in range(2)]
                                for kk in range(KK):
                                    xsl = xq[kk // 4][:, kk % 4, :]
                                    st, sp = kk == 0, kk == KK - 1
                                    for jj in range(2):
                                        tt = half * 2 + jj
                                        nc.tensor.matmul(
                                            pv[jj][:],
                                            xsl[:, tt * 128 : (tt + 1) * 128],
                                            wv_sb[:, kk, :],
                                            start=st,
                                            stop=sp,
                                        )
                                for jj in range(2):
                                    nc.vector.tensor_copy(
                                        v_sb[:, strip * 4 + half * 2 + jj, :], pv[jj][:]
                                    )

                            # attention for strip s of both local heads of batch b
                            for hl in range(2):
                                attention_unit(tc, nc, a2a_in, qT, kT, v_sb, ones,
                                               masks, expp, smallp, otp, psT, psA,
                                               psS, b, hl, s)
                                if b == 1 and s == 3 and hl == 0:
                                    # all h0 units done -> re-shard them while
                                    # the last h1 unit computes
                                    nc.gpsimd.collective_compute(
                                        "AllToAll", mybir.AluOpType.bypass,
                                        replica_groups=[list(range(N_CORES))],
                                        ins=[a2a_in[0][:].opt()],
                                        outs=[a2a_out[0][:].opt()],
                                    )

                    nc.gpsimd.collective_compute(
                        "AllToAll", mybir.AluOpType.bypass,
                        replica_groups=[list(range(N_CORES))],
                        ins=[a2a_in[1][:].opt()],
                        outs=[a2a_out[1][:].opt()],
                    )

                    # ---------------- Phase 4: output projection ----------------
                    ot_sb = [persist.tile([128, 8, 512], BF, name=f"ot_sb{h}")
                             for h in range(2)]
                    for h in range(2):
                        nc.sync.dma_start(
                            ot_sb[h][:], a2a_out[h][:].rearrange("i f t -> f i t")
                        )
                    for dd in range(KK):
                        op = ps1.tile([128, 512], FP, tag="p10", name=f"op{dd}")
                        for h in range(2):
                            for i in range(8):
                                nc.tensor.matmul(
                                    op[:],
                                    wo_tiles[dd][:, 2 * i + h, :],
                                    ot_sb[h][:, i, :],
                                    start=(h == 0 and i == 0),
                                    stop=(h == 1 and i == 7),
                                )
                        ob = otp.tile([128, 512], FP, tag="ob")
                        nc.scalar.copy(ob[:], op[:])
                        nc.sync.dma_start(out_t[dd * 128 : (dd + 1) * 128, :], ob[:])

    nc.compile()
    return nc


def attention_unit(tc, nc, a2a_in, qT, kT, v_sb, ones, masks,
                   expp, smallp, otp, psT, psA, psS, b, hl, s):
    q0 = b * S + s * 512
    qts = qT[:, hl, q0 : q0 + 512]
    avp = psA.tile([128, 512], FP, tag="av")
    smp = psS.tile([1, 512], FP, tag="sm")
    nk = 4 * (s + 1)
    for ki in range(nk):
        stp = psT.tile([128, 512], FP, tag="st")
        nc.tensor.matmul(
            stp[:],
            kT[:, hl, b * S + ki * 128 : b * S + (ki + 1) * 128],
            qts,
            start=True,
            stop=True,
        )
        if ki >= 4 * s:
            nc.vector.tensor_add(stp[:], stp[:], masks[:, ki - 4 * s, :])
        ex = expp.tile([128, 512], BF, tag="ex")
        nc.scalar.activation(ex[:], stp[:], mybir.ActivationFunctionType.Exp)
        st, sp = ki == 0, ki == nk - 1
        nc.tensor.matmul(
            avp[:],
            v_sb[:, b * 16 + ki, hl * 128 : (hl + 1) * 128],
            ex[:],
            start=st,
            stop=sp,
        )
        nc.tensor.matmul(smp[:], ones[:], ex[:], start=st, stop=sp)
    sums_sb = smallp.tile([1, 512], FP, tag="sums")
    nc.scalar.copy(sums_sb[:], smp[:])
    sbc = smallp.tile([128, 512], FP, tag="sbc")
    nc.gpsimd.partition_broadcast(sbc[:], sums_sb[:])
    rbc = smallp.tile([128, 512], FP, tag="rbc")
    nc.vector.reciprocal_approx_fast(rbc[:], sbc[:])
    ot = otp.tile([128, 512], BF, tag="ot")
    nc.vector.tensor_mul(ot[:], avp[:], rbc[:])
    j = b * 4 + s
    nc.sync.dma_start(a2a_in[hl][j, :, :], ot[:])


_NC_CACHE = {}


def _get_nc(debug_taps=False):
    key = bool(debug_taps)
    if key not in _NC_CACHE:
        _NC_CACHE[key] = build_nc(debug_taps=key)
    return _NC_CACHE[key]


def _make_in_maps(x, wq, wk, wv, wo):
    x = np.ascontiguousarray(np.asarray(x, dtype=np.float32))
    wq = np.asarray(wq, dtype=np.float32)
    wk = np.asarray(wk, dtype=np.float32)
    wv = np.asarray(wv, dtype=np.float32)
    wo = np.asarray(wo, dtype=np.float32)

    x_flat = x.reshape(T, D)
    # xt[p, kk, t] = x_flat[t, kk*128+p]
    xt_host = np.ascontiguousarray(
        x_flat.T.reshape(KK, 128, T).transpose(1, 0, 2)
    ).astype(NPBF)
    # wo_dev[p, ff, d] = wo[d, ff*128+p]
    wo_host = np.ascontiguousarray(
        wo.T.reshape(KK, 128, D).transpose(1, 0, 2)
    ).astype(NPBF)
    scale = 1.0 / np.sqrt(np.float32(DH))

    in_maps = []
    for c in range(N_CORES):
        sl = slice(c * 256, (c + 1) * 256)

        def wslice(w, scaled=False):
            wc = w[sl, :].T  # [D, 256]
            if scaled:
                wc = wc * scale
            return np.ascontiguousarray(
                wc.reshape(KK, 128, 256).transpose(1, 0, 2)
            ).astype(NPBF)

        in_maps.append(
            {
                "xt": xt_host,
                "wq": wslice(wq, scaled=True),
                "wk": wslice(wk),
                "wv": wslice(wv),
                "wo": wo_host,
            }
        )
    return in_maps


def _run(x, wq, wk, wv, wo, trace=False):
    nc = _get_nc()
    in_maps = _make_in_maps(x, wq, wk, wv, wo)
    res = run_bass_kernel_spmd(nc, in_maps, list(range(N_CORES)), trace=trace)
    rows = [res.results[c]["out_t"].T for c in range(N_CORES)]  # [512, D] each
    out = np.concatenate(rows, axis=0).reshape(B, S, D)
    return out, res


def kernel(x, wq, wk, wv, wo):
    out, _ = _run(x, wq, wk, wv, wo, trace=False)
    return out



# revision 5
# speedup vs baseline: 1.1132x; 1.1132x over previous
"""Context-parallel causal attention block on 8 Trainium2 NeuronCores.

Strategy: tensor-parallel split-heads. Each core c computes Q/K/V projections
for its 2 heads (of 16) over all tokens with host-sliced weights, runs causal
attention locally (feature-major layouts, no transposes), then one on-device
AllToAll re-shards from head-parallel to token-parallel, and each core runs the
output projection for its 512-token row slice. Host concatenates row slices.

Schedule (v2): projections interleave with head-0 attention only; AllToAll#0
fires right after the last head-0 unit and is covered by all head-1 attention;
AllToAll#1 is covered by the h0-feature half of the output projection (2-pass
oproj with an fp32 SBUF accumulator). Softmax row-sums are computed with one
ones-matmul per 4-block group (DVE pre-adds the exp tiles), and diagonal
blocks stream only their causally-active columns.

Matmul operands are bf16 (fp32 matmuls are two-pass / half-rate on TRN2's PE);
all accumulation stays fp32 in PSUM, softmax runs on fp32 scores.
"""
import sys

sys.path.insert(0, "/opt/trn_rl_repo")

import ml_dtypes
import numpy as np

import concourse.bass as bass
import concourse.tile as tile
from concourse import bacc, mybir
from concourse.bass_utils import run_bass_kernel_spmd

FP = mybir.dt.float32
BF = mybir.dt.bfloat16
NPBF = ml_dtypes.bfloat16
N_CORES = 8
B, S, D, H, DH = 2, 2048, 2048, 16, 128
T = B * S            # 4096 flattened tokens, b-major
KK = D // 128        # 16 contraction k-tiles
NSTRIP = T // 512    # 8 token strips of 512
ROWS = T // N_CORES  # 512 output rows per core
NEG = -1.0e30


def build_nc() -> bacc.Bacc:
    nc = bacc.Bacc("TRN2", target_bir_lowering=False, debug=False, num_devices=N_CORES)

    xt = nc.dram_tensor("xt", [128, KK, T], BF, kind="ExternalInput")
    wq = nc.dram_tensor("wq", [128, KK, 256], BF, kind="ExternalInput")
    wk = nc.dram_tensor("wk", [128, KK, 256], BF, kind="ExternalInput")
    wv = nc.dram_tensor("wv", [128, KK, 256], BF, kind="ExternalInput")
    # wo laid out host-side as [p, h, i, d]: feature tile (2i+h), out col d
    wo = nc.dram_tensor("wo", [128, 2, 8, D], BF, kind="ExternalInput")
    out_t = nc.dram_tensor("out_t", [D, ROWS], FP, kind="ExternalOutput")

    with tile.TileContext(nc) as tc:
        with (
            tc.tile_pool(name="dram", bufs=1, space="DRAM") as dram,
            tc.tile_pool(name="consts", bufs=1) as consts,
            tc.tile_pool(name="persist", bufs=1) as persist,
        ):
            a2a_in = [dram.tile([N_CORES, 128, 512], BF, name=f"a2a_in{h}") for h in range(2)]
            a2a_out = [dram.tile([N_CORES, 128, 512], BF, name=f"a2a_out{h}") for h in range(2)]
            wu_in = dram.tile([N_CORES, 1, 8], BF, name="wu_in")
            wu_out = dram.tile([N_CORES, 1, 8], BF, name="wu_out")

            ones = consts.tile([128, 1], BF)
            nc.gpsimd.memset(ones[:], 1.0)
            wu_sb = consts.tile([1, 64], BF)
            nc.gpsimd.memset(wu_sb[:], 0.0)
            nc.sync.dma_start(wu_in[:].rearrange("i o t -> o (i t)"), wu_sb[:])
            # warm up the collective stream so the real AllToAlls start fast
            nc.gpsimd.collective_compute(
                "AllToAll", mybir.AluOpType.bypass,
                replica_groups=[list(range(N_CORES))],
                ins=[wu_in[:].opt()],
                outs=[wu_out[:].opt()],
            )
            # additive causal masks for the 4 diagonal offsets:
            # masks[p, i, q] = 0 if q >= p + i*128 else NEG
            masks = consts.tile([128, 4, 512], FP)
            nc.gpsimd.memset(masks[:], 0.0)
            for i in range(4):
                nc.gpsimd.affine_select(
                    out=masks[:, i, :],
                    in_=masks[:, i, :],
                    compare_op=mybir.AluOpType.is_ge,
                    fill=NEG,
                    base=-(i * 128),
                    pattern=[[1, 512]],
                    channel_multiplier=-1,
                )

            qT = persist.tile([128, 2, T], BF)       # [dh, hl, t]
            kT = persist.tile([128, 2, T], BF)
            v_sb = persist.tile([128, 32, 256], BF)  # [t%128, t//128, head_feat]
            acc0 = persist.tile([128, KK, 512], FP)  # oproj h0-pass partials
            ot_sb = [persist.tile([128, 8, 512], BF, name=f"ot_sb{h}") for h in range(2)]

            with (
                tc.tile_pool(name="wpool", bufs=1) as wpool,
                tc.tile_pool(name="xtp", bufs=6) as xtp,
                tc.tile_pool(name="ps1", bufs=2, space="PSUM") as ps1,
                tc.tile_pool(name="expp", bufs=5) as expp,
                tc.tile_pool(name="gp", bufs=2) as gp,
                tc.tile_pool(name="smallp", bufs=2) as smallp,
                tc.tile_pool(name="otp", bufs=3) as otp,
                tc.tile_pool(name="wop", bufs=4) as wop,
                tc.tile_pool(name="psT", bufs=2, space="PSUM") as psT,
                tc.tile_pool(name="psA", bufs=1, space="PSUM") as psA,
                tc.tile_pool(name="psS", bufs=1, space="PSUM") as psS,
            ):
                wq_sb = wpool.tile([128, KK, 256], BF)
                wk_sb = wpool.tile([128, KK, 256], BF)
                wv_sb = wpool.tile([128, KK, 256], BF)
                nc.sync.dma_start(wq_sb[:], wq[:])

                # ---- Phase 1+2: projections + head-0 attention, interleaved ----
                for b in range(B):
                    for s in range(4):
                        strip = b * 4 + s
                        t0 = strip * 512
                        xq = []
                        for qtr in range(4):
                            xtile = xtp.tile([128, 4, 512], BF, tag="xt")
                            nc.sync.dma_start(
                                xtile[:],
                                xt[:, qtr * 4 : (qtr + 1) * 4, t0 : t0 + 512],
                            )
                            xq.append(xtile)
                        if b == 0 and s == 0:
                            # k/v weights load behind strip-0 x so the q-pass
                            # starts as early as possible
                            nc.sync.dma_start(wk_sb[:], wk[:])
                            nc.sync.dma_start(wv_sb[:], wv[:])

                        # pass A: q for both heads (2 banks)
                        pa = [ps1.tile([128, 512], FP, tag=f"p1{j}", name=f"pa{j}")
                              for j in range(2)]
                        for kk in range(KK):
                            xsl = xq[kk // 4][:, kk % 4, :]
                            st, sp = kk == 0, kk == KK - 1
                            nc.tensor.matmul(pa[0][:], wq_sb[:, kk, 0:128], xsl, start=st, stop=sp)
                            nc.tensor.matmul(pa[1][:], wq_sb[:, kk, 128:256], xsl, start=st, stop=sp)
                        for hl in range(2):
                            nc.scalar.copy(qT[:, hl, t0 : t0 + 512], pa[hl][:])
                        # pass B: k for both heads
                        pb = [ps1.tile([128, 512], FP, tag=f"p1{j}", name=f"pb{j}")
                              for j in range(2)]
                        for kk in range(KK):
                            xsl = xq[kk // 4][:, kk % 4, :]
                            st, sp = kk == 0, kk == KK - 1
                            nc.tensor.matmul(pb[0][:], wk_sb[:, kk, 0:128], xsl, start=st, stop=sp)
                            nc.tensor.matmul(pb[1][:], wk_sb[:, kk, 128:256], xsl, start=st, stop=sp)
                        for hl in range(2):
                            nc.scalar.copy(kT[:, hl, t0 : t0 + 512], pb[hl][:])
                        # pass C/D: v in two tt sub-passes (2 banks each)
                        for half in range(2):
                            pv = [ps1.tile([128, 256], FP, tag=f"p1{j}", name=f"pv{j}")
                                  for j in range(2)]
                            for kk in range(KK):
                                xsl = xq[kk // 4][:, kk % 4, :]
                                st, sp = kk == 0, kk == KK - 1
                                for jj in range(2):
                                    tt = half * 2 + jj
                                    nc.tensor.matmul(
                                        pv[jj][:],
                                        xsl[:, tt * 128 : (tt + 1) * 128],
                                        wv_sb[:, kk, :],
                                        start=st,
                                        stop=sp,
                                    )
                            for jj in range(2):
                                nc.vector.tensor_copy(
                                    v_sb[:, strip * 4 + half * 2 + jj, :], pv[jj][:]
                                )

                        attention_unit(tc, nc, a2a_in, qT, kT, v_sb, ones, masks,
                                       expp, gp, smallp, otp, psT, psA, psS, b, 0, s)

                nc.gpsimd.collective_compute(
                    "AllToAll", mybir.AluOpType.bypass,
                    replica_groups=[list(range(N_CORES))],
                    ins=[a2a_in[0][:].opt()],
                    outs=[a2a_out[0][:].opt()],
                )
                nc.sync.dma_start(
                    ot_sb[0][:], a2a_out[0][:].rearrange("i f t -> f i t")
                )

                # ---- Phase 3: head-1 attention (covers AllToAll#0) ----
                for b in range(B):
                    for s in range(4):
                        attention_unit(tc, nc, a2a_in, qT, kT, v_sb, ones, masks,
                                       expp, gp, smallp, otp, psT, psA, psS, b, 1, s)

                nc.gpsimd.collective_compute(
                    "AllToAll", mybir.AluOpType.bypass,
                    replica_groups=[list(range(N_CORES))],
                    ins=[a2a_in[1][:].opt()],
                    outs=[a2a_out[1][:].opt()],
                )
                nc.sync.dma_start(
                    ot_sb[1][:], a2a_out[1][:].rearrange("i f t -> f i t")
                )

                # ---- Phase 4: output projection, two passes ----
                # pass 0 (h0 features) overlaps AllToAll#1; pass 1 adds h1.
                for h in range(2):
                    for dd in range(KK):
                        wod = wop.tile([128, 8, 128], BF, tag="wod")
                        nc.sync.dma_start(wod[:], wo[:, h, :, dd * 128 : (dd + 1) * 128])
                        op = ps1.tile([128, 512], FP, tag="p10", name=f"op{h}_{dd}")
                        for i in range(8):
                            nc.tensor.matmul(
                                op[:],
                                wod[:, i, :],
                                ot_sb[h][:, i, :],
                                start=(i == 0),
                                stop=(i == 7),
                            )
                        if h == 0:
                            nc.scalar.copy(acc0[:, dd, :], op[:])
                        else:
                            ob = otp.tile([128, 512], FP, tag="ob")
                            nc.vector.tensor_add(ob[:], op[:], acc0[:, dd, :])
                            nc.sync.dma_start(out_t[dd * 128 : (dd + 1) * 128, :], ob[:])

    nc.compile()
    return nc


def attention_unit(tc, nc, a2a_in, qT, kT, v_sb, ones, masks,
                   expp, gp, smallp, otp, psT, psA, psS, b, hl, s):
    q0 = b * S + s * 512
    qts = qT[:, hl, q0 : q0 + 512]
    avp = psA.tile([128, 512], FP, tag="av")
    smp = psS.tile([1, 512], FP, tag="sm")
    nk = 4 * (s + 1)
    ngroups = s + 1
    for g in range(ngroups):
        diag = g == s
        exs = []
        for j in range(4):
            ki = g * 4 + j
            lo = j * 128 if diag else 0  # causally-active columns start here
            stp = psT.tile([128, 512], FP, tag="st")
            nc.tensor.matmul(
                stp[:, lo:],
                kT[:, hl, b * S + ki * 128 : b * S + (ki + 1) * 128],
                qts[:, lo:],
                start=True,
                stop=True,
            )
            if diag:
                nc.vector.tensor_add(stp[:, lo:], stp[:, lo:], masks[:, j, lo:])
            ex = expp.tile([128, 512], BF, tag="ex")
            nc.scalar.activation(ex[:, lo:], stp[:, lo:], mybir.ActivationFunctionType.Exp)
            nc.tensor.matmul(
                avp[:, lo:],
                v_sb[:, b * 16 + ki, hl * 128 : (hl + 1) * 128],
                ex[:, lo:],
                start=(ki == 0),
                stop=(ki == nk - 1),
                skip_group_check=True,
            )
            exs.append(ex)
        # per-group exp-sum: DVE folds 4 tiles, one ones-matmul per group
        gacc = gp.tile([128, 512], BF, tag="g")
        if diag:
            nc.vector.tensor_copy(gacc[:], exs[0][:])
            for j in range(1, 4):
                lo = j * 128
                nc.vector.tensor_add(gacc[:, lo:], gacc[:, lo:], exs[j][:, lo:])
        else:
            t01 = gp.tile([128, 512], BF, tag="t01")
            nc.vector.tensor_add(t01[:], exs[0][:], exs[1][:])
            nc.vector.tensor_add(gacc[:], exs[2][:], exs[3][:])
            nc.vector.tensor_add(gacc[:], gacc[:], t01[:])
        nc.tensor.matmul(smp[:], ones[:], gacc[:],
                         start=(g == 0), stop=(g == ngroups - 1))
    sums_sb = smallp.tile([1, 512], FP, tag="sums")
    nc.scalar.copy(sums_sb[:], smp[:])
    sbc = smallp.tile([128, 512], FP, tag="sbc")
    nc.gpsimd.partition_broadcast(sbc[:], sums_sb[:])
    rbc = smallp.tile([128, 512], FP, tag="rbc")
    nc.vector.reciprocal_approx_fast(rbc[:], sbc[:])
    ot = otp.tile([128, 512], BF, tag="ot")
    nc.vector.tensor_mul(ot[:], avp[:], rbc[:])
    j = b * 4 + s
    nc.sync.dma_start(a2a_in[hl][j, :, :], ot[:])


_NC_CACHE = {}


def _get_nc():
    if "nc" not in _NC_CACHE:
        _NC_CACHE["nc"] = build_nc()
    return _NC_CACHE["nc"]


def _make_in_maps(x, wq, wk, wv, wo):
    x = np.ascontiguousarray(np.asarray(x, dtype=np.float32))
    wq = np.asarray(wq, dtype=np.float32)
    wk = np.asarray(wk, dtype=np.float32)
    wv = np.asarray(wv, dtype=np.float32)
    wo = np.asarray(wo, dtype=np.float32)

    x_flat = x.reshape(T, D)
    # xt[p, kk, t] = x_flat[t, kk*128+p]
    xt_host = np.ascontiguousarray(
        x_flat.T.reshape(KK, 128, T).transpose(1, 0, 2)
    ).astype(NPBF)
    # wo_dev[p, h, i, d] = wo[d, (2i+h)*128+p]
    wo_host = np.ascontiguousarray(
        wo.T.reshape(8, 2, 128, D).transpose(2, 1, 0, 3)
    ).astype(NPBF)
    scale = 1.0 / np.sqrt(np.float32(DH))

    in_maps = []
    for c in range(N_CORES):
        sl = slice(c * 256, (c + 1) * 256)

        def wslice(w, scaled=False):
            wc = w[sl, :].T  # [D, 256]
            if scaled:
                wc = wc * scale
            return np.ascontiguousarray(
                wc.reshape(KK, 128, 256).transpose(1, 0, 2)
            ).astype(NPBF)

        in_maps.append(
            {
                "xt": xt_host,
                "wq": wslice(wq, scaled=True),
                "wk": wslice(wk),
                "wv": wslice(wv),
                "wo": wo_host,
            }
        )
    return in_maps


def _run(x, wq, wk, wv, wo, trace=False):
    nc = _get_nc()
    in_maps = _make_in_maps(x, wq, wk, wv, wo)
    res = run_bass_kernel_spmd(nc, in_maps, list(range(N_CORES)), trace=trace)
    rows = [res.results[c]["out_t"].T for c in range(N_CORES)]  # [512, D] each
    out = np.concatenate(rows, axis=0).reshape(B, S, D)
    return out, res


def kernel(x, wq, wk, wv, wo):
    out, _ = _run(x, wq, wk, wv, wo, trace=False)
    return out
